# revision 4
# baseline (speedup 1.0000x reference)
"""Multi-head attention (B=2, S=2048, D=1024, H=16, causal, interleaved RoPE)
on 8 Trainium2 NeuronCores.

Sharding: tensor-parallel over heads — 2 heads (128 channels) per core.
Each core computes its Q/K/V projections, RoPE, causal attention, and a
row-parallel partial of the output projection; the host sums the partials.

All matmuls run as fp32r (TF32-like, full PE rate at free-dim >= 256).

Layout tricks:
  * Q/K projection weights are fed with output channels permuted so each
    head's dims are [evens(32), odds(32)] -> RoPE pair-swap becomes a
    32-partition-block swap done with SBUF->SBUF DMAs.
  * Attention uses the S^T layout: scores psum [k(128part), q(512)] via
    matmul(lhsT=K^T, rhs=Q^T); softmax needs no max subtraction (scores
    are O(10) here); exp on ACT; causal mask = multiply by 0/1 slice of a
    precomputed [128,1024] band tile; PV via matmul(lhsT=V_aug, rhs=P^T)
    where V_aug carries a ones column so the denominator drops out as psum
    row 64; 1/denom is broadcast across partitions with a K=1 matmul and
    applied on the PV psum -> y^T copy.
  * x^T (contraction over D needs D on partitions) is produced on-device
    with PE transposes of natural x tiles.
"""

import numpy as np

import concourse.bacc as bacc
import concourse.mybir as mybir
import concourse.tile as tile
from concourse.bass_utils import run_bass_kernel_spmd
from concourse.masks import make_identity

P = 128
B, S, D = 2, 2048, 1024
H, DH = 16, 64
NROWS = B * S            # 4096 flattened rows
CH = 128                 # channels per core (2 heads)
RB = 512                 # row block for projections / q tiles
NRB = NROWS // RB        # 8
DSUB = D // P            # 8 contraction subtiles
KSUB = NROWS // P        # 32 k subtiles (128 rows each)
QT_PER_B = S // RB       # 4 q tiles per batch
ROPE_BASE = 10000.0

f32 = mybir.dt.float32
f32r = mybir.dt.float32r

_CACHE = {}


import os
PHASES = os.environ.get("KPHASE", "ABCD")


def _build():
    nc = bacc.Bacc("TRN2", target_bir_lowering=False)

    x_ext = nc.declare_dram_parameter("x", [NROWS, D], f32r, isOutput=False)
    wqT_ext = nc.declare_dram_parameter("wqT", [D, CH], f32r, isOutput=False)
    wkT_ext = nc.declare_dram_parameter("wkT", [D, CH], f32r, isOutput=False)
    wvT_ext = nc.declare_dram_parameter("wvT", [D, CH], f32r, isOutput=False)
    woT_ext = nc.declare_dram_parameter("woT", [CH, D], f32r, isOutput=False)
    bq_ext = nc.declare_dram_parameter("bq", [CH, 1], f32, isOutput=False)
    bk_ext = nc.declare_dram_parameter("bk", [CH, 1], f32, isOutput=False)
    bv_ext = nc.declare_dram_parameter("bv", [CH, 1], f32, isOutput=False)
    cc_ext = nc.declare_dram_parameter("cc", [P, NROWS], f32r, isOutput=False)
    ss_ext = nc.declare_dram_parameter("ss", [P, NROWS], f32r, isOutput=False)
    mask_ext = nc.declare_dram_parameter("mask", [P, 1024], f32r, isOutput=False)
    out_ext = nc.declare_dram_parameter("out", [NROWS, D], f32, isOutput=True)

    with tile.TileContext(nc) as tc:
        with (
            tc.tile_pool(name="const", bufs=1) as cpool,
            tc.tile_pool(name="big", bufs=1) as big,
            tc.tile_pool(name="work", bufs=2) as work,
            tc.tile_pool(name="small", bufs=3) as small,
            tc.tile_pool(name="psumA", bufs=2, space="PSUM") as psumA,
            tc.tile_pool(name="psumB", bufs=1, space="PSUM") as psumB,
        ):
            # ---- constants ----
            ident_f = cpool.tile([P, P], f32, tag="identf")
            make_identity(nc, ident_f[:])
            ident = cpool.tile([P, P], f32r, tag="ident")
            nc.vector.tensor_copy(ident[:], ident_f[:])

            ones_f = cpool.tile([P, 64], f32, tag="onesf")
            nc.vector.memset(ones_f[:], 1.0)
            ones_r = cpool.tile([P, 64], f32r, tag="onesr")
            nc.vector.tensor_copy(ones_r[:], ones_f[:])

            wq_sb = cpool.tile([P, DSUB, CH], f32r, tag="wq")
            wk_sb = cpool.tile([P, DSUB, CH], f32r, tag="wk")
            wv_sb = cpool.tile([P, DSUB, CH], f32r, tag="wv")
            for d in range(DSUB):
                nc.sync.dma_start(wq_sb[:, d], wqT_ext[d * P:(d + 1) * P, :])
                nc.sync.dma_start(wk_sb[:, d], wkT_ext[d * P:(d + 1) * P, :])
                nc.sync.dma_start(wv_sb[:, d], wvT_ext[d * P:(d + 1) * P, :])
            wo_sb = cpool.tile([CH, D], f32r, tag="wo")
            nc.sync.dma_start(wo_sb[:, 0:512], woT_ext[:, 0:512])
            nc.sync.dma_start(wo_sb[:, 512:1024], woT_ext[:, 512:1024])
            bq_sb = cpool.tile([CH, 1], f32, tag="bq")
            nc.sync.dma_start(bq_sb[:], bq_ext[:])
            bk_sb = cpool.tile([CH, 1], f32, tag="bk")
            nc.sync.dma_start(bk_sb[:], bk_ext[:])
            bv_sb = cpool.tile([CH, 1], f32, tag="bv")
            nc.sync.dma_start(bv_sb[:], bv_ext[:])
            mask_sb = cpool.tile([P, 1024], f32r, tag="mask")
            nc.sync.dma_start(mask_sb[:], mask_ext[:])

            # ---- persistent activation tiles ----
            qT = big.tile([P, NROWS], f32r, tag="qT")     # roped Q^T (pre-scaled 1/8)
            kT = big.tile([P, NROWS], f32r, tag="kT")     # roped K^T
            yT = big.tile([P, NROWS], f32r, tag="yT")     # attention out ^T
            v_sb = big.tile([P, KSUB, 130], f32r, tag="v")  # V natural + ones cols

            # ones columns of v (cols 64 and 129 of each k-subtile)
            nc.vector.tensor_copy(
                v_sb[:, :, 64:130:65].rearrange("p a b -> p (a b)"),
                ones_r[:, 0:2 * KSUB])

            # ================= phase A: x^T, projections =================
            proj_list = [
                ("q", wq_sb, bq_sb, 0.125, qT),
                ("k", wk_sb, bk_sb, 1.0, kT),
                ("v", wv_sb, bv_sb, 1.0, None),
            ]
            for rt in range(NRB):               # 8 blocks of 512 rows
                xT = work.tile([P, DSUB, RB], f32r, tag="xT")
                for rc in range(RB // P):       # 4 chunks of 128 rows
                    r0 = rt * RB + rc * P
                    xa = work.tile([P, D], f32r, tag="xa")
                    nc.sync.dma_start(xa[:], x_ext[r0:r0 + P, :])
                    for half in range(2):
                        tp = psumA.tile([P, 512], f32r, tag="tp")
                        for j in range(4):
                            d = half * 4 + j
                            nc.tensor.transpose(
                                tp[:, j * P:(j + 1) * P],
                                xa[:, d * P:(d + 1) * P], ident[:])
                        # scatter 4 transposed d-subtiles into xT
                        nc.vector.tensor_copy(
                            xT[:, half * 4:(half + 1) * 4, rc * P:(rc + 1) * P],
                            tp[:].rearrange("p (j c) -> p j c", j=4))

                for name, w_sb, b_sb, scale, dstT in proj_list:
                    pp = psumA.tile([P, RB], f32, tag="proj")
                    for d in range(DSUB):
                        nc.tensor.matmul(pp[:], w_sb[:, d], xT[:, d],
                                         start=(d == 0), stop=(d == DSUB - 1))
                    if name != "v":
                        nc.scalar.activation(
                            dstT[:, rt * RB:(rt + 1) * RB], pp[:],
                            mybir.ActivationFunctionType.Identity,
                            bias=b_sb[:, 0:1], scale=scale)
                    else:
                        # V^T chunk with bias, then PE-transpose to natural V
                        vr = work.tile([P, RB], f32r, tag="vraw")
                        nc.scalar.activation(
                            vr[:], pp[:],
                            mybir.ActivationFunctionType.Identity,
                            bias=b_sb[:, 0:1], scale=1.0)
                        tpv = psumA.tile([P, 512], f32r, tag="tp")
                        for rc2 in range(4):
                            nc.tensor.transpose(
                                tpv[:, rc2 * P:(rc2 + 1) * P],
                                vr[:, rc2 * P:(rc2 + 1) * P], ident[:])
                        # scatter: head0 chans -> cols 0:64, head1 -> cols 65:129
                        tpv_v = tpv[:].rearrange("p (k h c) -> p k h c", k=4, h=2)
                        vdst = (v_sb[:, rt * 4:(rt + 1) * 4, 0:130]
                                .rearrange("p k (h c) -> p k h c", h=2))
                        for hh in range(2):
                            nc.vector.tensor_copy(vdst[:, :, hh, 0:64],
                                                  tpv_v[:, :, hh, :])

            # ============ phase B: RoPE on qT, kT (in place) ============
            for xt in ((qT, kT) if "B" in PHASES else ()):
                for ch in range(NRB):
                    sl = slice(ch * RB, (ch + 1) * RB)
                    xsw = work.tile([P, RB], f32r, tag="xsw")
                    for hh in range(2):
                        b0 = hh * 64
                        nc.sync.dma_start(xsw[b0:b0 + 32, :], xt[b0 + 32:b0 + 64, sl])
                        nc.sync.dma_start(xsw[b0 + 32:b0 + 64, :], xt[b0:b0 + 32, sl])
                    ccc = small.tile([P, RB], f32r, tag="ccc")
                    nc.sync.dma_start(ccc[:], cc_ext[:, sl])
                    sss = small.tile([P, RB], f32r, tag="sss")
                    nc.sync.dma_start(sss[:], ss_ext[:, sl])
                    t1 = small.tile([P, RB], f32r, tag="ropet1")
                    nc.vector.tensor_mul(t1[:], xt[:, sl], ccc[:])
                    t2 = small.tile([P, RB], f32r, tag="ropet2")
                    nc.vector.tensor_mul(t2[:], xsw[:], sss[:])
                    nc.vector.tensor_add(xt[:, sl], t1[:], t2[:])

            # ================= phase C: attention =================
            if "C" not in PHASES:
                # dump qT as a stand-in output and stop
                for rt in range(NRB):
                    tq = small.tile([P, RB], f32, tag="ob")
                    nc.vector.tensor_copy(tq[:], qT[:, rt * RB:(rt + 1) * RB])
                    nc.sync.dma_start(out_ext[0:P, 0:RB], tq[:])
            for b in range(B if "C" in PHASES else 0):
                for h in range(2):
                    hsl = slice(h * 64, (h + 1) * 64)
                    for qt in range(QT_PER_B):
                        qcols = slice(b * S + qt * RB, b * S + (qt + 1) * RB)
                        nks = qt * 4 + 4
                        pv = psumB.tile([65, RB], f32, tag="pv")
                        for ks in range(nks):
                            st = psumA.tile([P, RB], f32, tag="st")
                            kcols = slice(b * S + ks * P, b * S + (ks + 1) * P)
                            nc.tensor.matmul(st[:], kT[hsl, kcols], qT[hsl, qcols],
                                             start=True, stop=True)
                            pt = small.tile([P, RB], f32r, tag="pt")
                            nc.scalar.activation(pt[:], st[:],
                                                 mybir.ActivationFunctionType.Exp)
                            m = ks - qt * 4
                            if m >= 0:
                                off = 512 - m * P
                                nc.vector.tensor_mul(pt[:], pt[:],
                                                     mask_sb[:, off:off + RB])
                            ksg = b * (S // P) + ks
                            nc.tensor.matmul(
                                pv[:], v_sb[:, ksg, h * 65:(h + 1) * 65], pt[:],
                                start=(ks == 0), stop=(ks == nks - 1))
                        # denominator row 64 -> reciprocal -> broadcast via K=1 mm
                        rcp = small.tile([65, RB], f32r, tag="rcp")
                        with nc.allow_low_precision(reason="f32r recip, 1e-4 ok"):
                            nc.vector.reciprocal(rcp[64:65, :], pv[64:65, :])
                        rep = psumB.tile([64, RB], f32, tag="rep")
                        nc.tensor.matmul(rep[:], ones_r[64:65, 0:64],
                                         rcp[64:65, :], start=True, stop=True)
                        rep_sb = small.tile([64, RB], f32, tag="repsb")
                        nc.scalar.copy(rep_sb[:], rep[:])
                        if h == 0:
                            nc.vector.tensor_mul(yT[0:64, qcols], pv[0:64, :],
                                                 rep_sb[:])
                        else:
                            t64 = small.tile([64, RB], f32r, tag="t64")
                            nc.vector.tensor_mul(t64[:], pv[0:64, :], rep_sb[:])
                            nc.sync.dma_start(yT[64:128, qcols], t64[:])

            # ================= phase D: output projection =================
            if "D" not in PHASES and "C" in PHASES:
                for qt2 in range(NRB):
                    ty = small.tile([P, RB], f32, tag="ob")
                    nc.vector.tensor_copy(ty[:], yT[:, qt2 * RB:(qt2 + 1) * RB])
                    nc.sync.dma_start(out_ext[0:P, 0:RB], ty[:])
            for rt in range(KSUB if "D" in PHASES else 0):              # 32 tiles of 128 rows
                for ec in range(2):
                    op = psumA.tile([P, 512], f32, tag="proj")
                    nc.tensor.matmul(op[:], yT[:, rt * P:(rt + 1) * P],
                                     wo_sb[:, ec * 512:(ec + 1) * 512],
                                     start=True, stop=True)
                    ob = small.tile([P, 512], f32, tag="ob")
                    nc.vector.tensor_copy(ob[:], op[:])
                    nc.sync.dma_start(
                        out_ext[rt * P:(rt + 1) * P, ec * 512:(ec + 1) * 512],
                        ob[:])

    nc.finalize()
    return nc


def _host_inputs():
    t = np.arange(32, dtype=np.float64)
    inv_freq = 1.0 / (ROPE_BASE ** (2.0 * t / DH))
    pos = np.arange(S, dtype=np.float64)
    ang = pos[None, :] * inv_freq[:, None]          # [32, S]
    cos32 = np.cos(ang).astype(np.float32)
    sin32 = np.sin(ang).astype(np.float32)
    cos32 = np.tile(cos32, (1, B))                  # [32, 4096]
    sin32 = np.tile(sin32, (1, B))
    cc = np.tile(cos32, (4, 1))                     # [128, 4096]
    ss = np.concatenate([-sin32, sin32, -sin32, sin32], axis=0)  # [128, 4096]

    ii = np.arange(P)[:, None]
    jj = np.arange(1024)[None, :]
    mask = (jj >= ii + 512).astype(np.float32)      # [128, 1024]

    perm64 = np.concatenate([np.arange(0, 64, 2), np.arange(1, 64, 2)])
    return cc, ss, mask, perm64


def kernel(x, Wq, bq, Wk, bk, Wv, bv, Wo, bo):
    x = np.asarray(x, dtype=np.float32)
    Wq = np.asarray(Wq, dtype=np.float32)
    Wk = np.asarray(Wk, dtype=np.float32)
    Wv = np.asarray(Wv, dtype=np.float32)
    Wo = np.asarray(Wo, dtype=np.float32)
    bq = np.asarray(bq, dtype=np.float32)
    bk = np.asarray(bk, dtype=np.float32)
    bv = np.asarray(bv, dtype=np.float32)
    bo = np.asarray(bo, dtype=np.float32)

    if "nc" not in _CACHE:
        _CACHE["nc"] = _build()
    nc = _CACHE["nc"]

    cc, ss, mask, perm64 = _host_inputs()
    x2 = np.ascontiguousarray(x.reshape(NROWS, D))
    perm128 = np.concatenate([perm64, perm64 + 64])

    in_maps = []
    for c in range(8):
        sl = slice(c * CH, (c + 1) * CH)
        wq_c = Wq[sl][perm128]                      # [128, 1024]
        wk_c = Wk[sl][perm128]
        wv_c = Wv[sl]
        in_maps.append({
            "x": x2,
            "wqT": np.ascontiguousarray(wq_c.T),
            "wkT": np.ascontiguousarray(wk_c.T),
            "wvT": np.ascontiguousarray(wv_c.T),
            "woT": np.ascontiguousarray(Wo[:, sl].T),
            "bq": (bq[sl][perm128] * 0.125).reshape(CH, 1).copy(),
            "bk": bk[sl][perm128].reshape(CH, 1).copy(),
            "bv": bv[sl].reshape(CH, 1).copy(),
            "cc": cc, "ss": ss, "mask": mask,
        })

    res = run_bass_kernel_spmd(nc, in_maps, core_ids=list(range(8)))
    out = np.zeros((NROWS, D), dtype=np.float32)
    for r in res.results:
        out += r["out"]
    out += bo[None, :]
    return out.reshape(B, S, D)


# revision 7
# speedup vs baseline: 1.3749x; 1.3749x over previous
"""Multi-head attention (B=2, S=2048, D=1024, H=16, causal, interleaved RoPE)
on 8 Trainium2 NeuronCores.

Sharding: tensor-parallel over heads — 2 heads (128 channels) per core.
Each core computes its Q/K/V projections, RoPE, causal attention, and a
row-parallel partial of the output projection; the host sums the partials.

Matmuls run in bf16 with fp32 PSUM accumulation (RoPE, softmax denominators
and all epilogues in fp32).

Layout:
  * Q/K projection weights are fed with output channels permuted so each
    head's dims are [evens(32), odds(32)] -> the RoPE pair-swap becomes a
    32-partition-block swap done with SBUF->SBUF DMAs; RoPE itself is three
    fp32 DVE multiplies/adds fused into the projection epilogue.
  * Attention uses the S^T layout: scores psum [k(128part), q(512)] via
    matmul(lhsT=K^T, rhs=Q^T), with the two heads issued back-to-back on
    disjoint PE row groups (partitions 0-63 / 64-127) so their weight loads
    overlap the other head's matmul. Softmax needs no max subtraction
    (scores are O(10)); exp on ACT writes bf16 P^T; causal masking is a
    multiply by a 0/1 slice of a [128,1024] band tile; PV via
    matmul(lhsT=V_aug, rhs=P^T) where V_aug carries a ones column so the
    denominator drops out as psum row 64; 1/denom (fast-approx reciprocal)
    is broadcast across partitions with a K=1 matmul and applied on the
    PV psum -> y^T copy.
  * x^T (contraction over D needs D on partitions) is produced on-device:
    cast to bf16 during the DMA (SWDGE), then PE transposes.
"""

import numpy as np
import ml_dtypes

import concourse.bacc as bacc
import concourse.mybir as mybir
import concourse.tile as tile
from concourse.bass_utils import run_bass_kernel_spmd
from concourse.masks import make_identity

P = 128
B, S, D = 2, 2048, 1024
H, DH = 16, 64
NROWS = B * S            # 4096 flattened rows
CH = 128                 # channels per core (2 heads)
RB = 512                 # row block for projections / q tiles
NRB = NROWS // RB        # 8
DSUB = D // P            # 8 contraction subtiles
KSUB = NROWS // P        # 32 k subtiles (128 rows each)
QT_PER_B = S // RB       # 4 q tiles per batch
ROPE_BASE = 10000.0

f32 = mybir.dt.float32
f32r = mybir.dt.float32r
bf16 = mybir.dt.bfloat16

_CACHE = {}


def _build():
    nc = bacc.Bacc("TRN2", target_bir_lowering=False)

    x_ext = nc.declare_dram_parameter("x", [NROWS, D], f32, isOutput=False)
    wqT_ext = nc.declare_dram_parameter("wqT", [D, CH], bf16, isOutput=False)
    wkT_ext = nc.declare_dram_parameter("wkT", [D, CH], bf16, isOutput=False)
    wvT_ext = nc.declare_dram_parameter("wvT", [D, CH], bf16, isOutput=False)
    woT_ext = nc.declare_dram_parameter("woT", [CH, D], bf16, isOutput=False)
    bq_ext = nc.declare_dram_parameter("bq", [CH, 1], f32, isOutput=False)
    bk_ext = nc.declare_dram_parameter("bk", [CH, 1], f32, isOutput=False)
    bv_ext = nc.declare_dram_parameter("bv", [CH, 1], f32, isOutput=False)
    cc_ext = nc.declare_dram_parameter("cc", [P, NROWS], f32, isOutput=False)
    ss_ext = nc.declare_dram_parameter("ss", [P, NROWS], f32, isOutput=False)
    mask_ext = nc.declare_dram_parameter("mask", [P, 1024], bf16, isOutput=False)
    out_ext = nc.declare_dram_parameter("out", [NROWS, D], f32, isOutput=True)

    with tile.TileContext(nc) as tc:
        with (
            tc.tile_pool(name="const", bufs=1) as cpool,
            tc.tile_pool(name="big", bufs=1) as big,
            tc.tile_pool(name="work", bufs=2) as work,
            tc.tile_pool(name="small", bufs=3) as small,
            tc.tile_pool(name="ptpool", bufs=6) as ptpool,
            tc.tile_pool(name="psumA", bufs=2, space="PSUM") as psumA,
            tc.tile_pool(name="psumB", bufs=1, space="PSUM") as psumB,
        ):
            # ---- constants ----
            ident_f = cpool.tile([P, P], f32, tag="identf")
            make_identity(nc, ident_f[:])
            ident = cpool.tile([P, P], bf16, tag="ident")
            nc.vector.tensor_copy(ident[:], ident_f[:])

            ones_f = cpool.tile([P, 64], f32, tag="onesf")
            nc.vector.memset(ones_f[:], 1.0)
            ones_b = cpool.tile([P, 64], bf16, tag="onesb")
            nc.vector.tensor_copy(ones_b[:], ones_f[:])
            ones_r = cpool.tile([P, 64], f32r, tag="onesr")
            nc.vector.tensor_copy(ones_r[:], ones_f[:])

            wq_sb = cpool.tile([P, DSUB, CH], bf16, tag="wq")
            wk_sb = cpool.tile([P, DSUB, CH], bf16, tag="wk")
            wv_sb = cpool.tile([P, DSUB, CH], bf16, tag="wv")
            for d in range(DSUB):
                nc.sync.dma_start(wq_sb[:, d], wqT_ext[d * P:(d + 1) * P, :])
                nc.sync.dma_start(wk_sb[:, d], wkT_ext[d * P:(d + 1) * P, :])
                nc.sync.dma_start(wv_sb[:, d], wvT_ext[d * P:(d + 1) * P, :])
            wo_sb = cpool.tile([CH, D], bf16, tag="wo")
            nc.sync.dma_start(wo_sb[:, 0:512], woT_ext[:, 0:512])
            nc.sync.dma_start(wo_sb[:, 512:1024], woT_ext[:, 512:1024])
            bq_sb = cpool.tile([CH, 1], f32, tag="bq")
            nc.sync.dma_start(bq_sb[:], bq_ext[:])
            bk_sb = cpool.tile([CH, 1], f32, tag="bk")
            nc.sync.dma_start(bk_sb[:], bk_ext[:])
            bv_sb = cpool.tile([CH, 1], f32, tag="bv")
            nc.sync.dma_start(bv_sb[:], bv_ext[:])
            mask_sb = cpool.tile([P, 1024], bf16, tag="mask")
            nc.sync.dma_start(mask_sb[:], mask_ext[:])

            # ---- persistent activation tiles ----
            qT = big.tile([P, NROWS], bf16, tag="qT")     # roped Q^T (pre-scaled 1/8)
            kT = big.tile([P, NROWS], bf16, tag="kT")     # roped K^T
            yT = big.tile([P, NROWS], bf16, tag="yT")     # attention out ^T
            v_sb = big.tile([P, KSUB, 130], bf16, tag="v")  # V natural + ones cols

            # ones columns of v (cols 64 and 129 of each k-subtile)
            nc.vector.tensor_copy(
                v_sb[:, :, 64:130:65].rearrange("p a b -> p (a b)"),
                ones_b[:, 0:2 * KSUB])

            # ====== phase A: x^T, projections, RoPE (fused per row block) ======
            proj_list = [
                ("q", wq_sb, bq_sb, 0.125, qT),
                ("k", wk_sb, bk_sb, 1.0, kT),
                ("v", wv_sb, bv_sb, 1.0, None),
            ]
            for rt in range(NRB):               # 8 blocks of 512 rows
                sl = slice(rt * RB, (rt + 1) * RB)
                xT = work.tile([P, DSUB, RB], bf16, tag="xT")
                for rc in range(RB // P):       # 4 chunks of 128 rows
                    r0 = rt * RB + rc * P
                    xab = work.tile([P, D], bf16, tag="xab")
                    # cast fp32 -> bf16 during the DMA (SWDGE)
                    nc.gpsimd.dma_start(xab[:], x_ext[r0:r0 + P, :])
                    for half in range(2):
                        tp = psumA.tile([P, 512], bf16, tag="tp")
                        for j in range(4):
                            d = half * 4 + j
                            nc.tensor.transpose(
                                tp[:, j * P:(j + 1) * P],
                                xab[:, d * P:(d + 1) * P], ident[:])
                        nc.vector.tensor_copy(
                            xT[:, half * 4:(half + 1) * 4, rc * P:(rc + 1) * P],
                            tp[:].rearrange("p (j c) -> p j c", j=4))

                ccc = small.tile([P, RB], f32, tag="ccc")
                nc.sync.dma_start(ccc[:], cc_ext[:, sl])
                sss = small.tile([P, RB], f32, tag="sss")
                nc.sync.dma_start(sss[:], ss_ext[:, sl])

                for name, w_sb, b_sb, scale, dstT in proj_list:
                    pp = psumA.tile([P, RB], f32, tag="proj")
                    for d in range(DSUB):
                        nc.tensor.matmul(pp[:], w_sb[:, d], xT[:, d],
                                         start=(d == 0), stop=(d == DSUB - 1))
                    if name != "v":
                        praw = work.tile([P, RB], f32, tag="praw")
                        nc.scalar.activation(
                            praw[:], pp[:],
                            mybir.ActivationFunctionType.Identity,
                            bias=b_sb[:, 0:1], scale=scale)
                        # RoPE: dst = praw*cc + swap32(praw)*ss  (fp32, ->bf16)
                        xsw = work.tile([P, RB], f32, tag="xsw")
                        for hh in range(2):
                            b0 = hh * 64
                            nc.sync.dma_start(xsw[b0:b0 + 32, :],
                                              praw[b0 + 32:b0 + 64, :])
                            nc.sync.dma_start(xsw[b0 + 32:b0 + 64, :],
                                              praw[b0:b0 + 32, :])
                        t1 = small.tile([P, RB], f32, tag="ropet1")
                        nc.vector.tensor_mul(t1[:], praw[:], ccc[:])
                        t2 = small.tile([P, RB], f32, tag="ropet2")
                        nc.vector.tensor_mul(t2[:], xsw[:], sss[:])
                        nc.vector.tensor_add(dstT[:, sl], t1[:], t2[:])
                    else:
                        # V^T chunk with bias, then PE-transpose to natural V
                        vr = work.tile([P, RB], bf16, tag="vraw")
                        nc.scalar.activation(
                            vr[:], pp[:],
                            mybir.ActivationFunctionType.Identity,
                            bias=b_sb[:, 0:1], scale=1.0)
                        tpv = psumA.tile([P, 512], bf16, tag="tp")
                        for rc2 in range(4):
                            nc.tensor.transpose(
                                tpv[:, rc2 * P:(rc2 + 1) * P],
                                vr[:, rc2 * P:(rc2 + 1) * P], ident[:])
                        # scatter: head0 chans -> cols 0:64, head1 -> cols 65:129
                        tpv_v = tpv[:].rearrange("p (k h c) -> p k h c", k=4, h=2)
                        vdst = (v_sb[:, rt * 4:(rt + 1) * 4, 0:130]
                                .rearrange("p k (h c) -> p k h c", h=2))
                        for hh in range(2):
                            nc.vector.tensor_copy(vdst[:, :, hh, 0:64],
                                                  tpv_v[:, :, hh, :])

            # ================= phase C: attention =================
            for b in range(B):
                for qt in range(QT_PER_B):
                    qcols = slice(b * S + qt * RB, b * S + (qt + 1) * RB)
                    nks = qt * 4 + 4
                    pvs = []
                    for h in range(2):
                        pv_t = psumB.tile([65, RB], f32, tag=f"pv{h}",
                                          name=f"pv{h}_{b}_{qt}")
                        pvs.append(pv_t)
                    for ks in range(nks):
                        kcols = slice(b * S + ks * P, b * S + (ks + 1) * P)
                        ksg = b * (S // P) + ks
                        m = ks - qt * 4
                        pts = []
                        for h in range(2):
                            hsl = slice(h * 64, (h + 1) * 64)
                            st = psumA.tile([P, RB], f32, tag="st")
                            nc.tensor.matmul(st[:], kT[hsl, kcols],
                                             qT[hsl, qcols],
                                             start=True, stop=True)
                            pt = ptpool.tile([P, RB], bf16, tag="pt")
                            nc.scalar.activation(pt[:], st[:],
                                                 mybir.ActivationFunctionType.Exp)
                            if m >= 0:
                                off = 512 - m * P
                                nc.vector.tensor_mul(pt[:], pt[:],
                                                     mask_sb[:, off:off + RB])
                            pts.append(pt)
                        for h in range(2):
                            nc.tensor.matmul(
                                pvs[h][:], v_sb[:, ksg, h * 65:(h + 1) * 65],
                                pts[h][:],
                                start=(ks == 0), stop=(ks == nks - 1))
                    for h in range(2):
                        pv = pvs[h]
                        rcp_f = small.tile([65, RB], f32, tag="rcpf")
                        with nc.allow_low_precision(reason="recip probe"):
                            nc.vector.reciprocal(rcp_f[64:65, :], pv[64:65, :])
                        rcp_r = small.tile([65, RB], f32r, tag="rcpr")
                        nc.vector.tensor_copy(rcp_r[64:65, :], rcp_f[64:65, :])
                        rep = psumA.tile([64, RB], f32, tag="tp")
                        nc.tensor.matmul(rep[:], ones_r[64:65, 0:64],
                                         rcp_r[64:65, :], start=True, stop=True)
                        rep_sb = small.tile([64, RB], f32, tag="repsb")
                        nc.scalar.copy(rep_sb[:], rep[:])
                        if h == 0:
                            nc.vector.tensor_mul(yT[0:64, qcols], pv[0:64, :],
                                                 rep_sb[:])
                        else:
                            t64 = small.tile([64, RB], bf16, tag="t64")
                            nc.vector.tensor_mul(t64[:], pv[0:64, :], rep_sb[:])
                            nc.sync.dma_start(yT[64:128, qcols], t64[:])

            # ================= phase D: output projection =================
            for rt in range(KSUB):              # 32 tiles of 128 rows
                for ec in range(2):
                    op = psumA.tile([P, 512], f32, tag="proj")
                    nc.tensor.matmul(op[:], yT[:, rt * P:(rt + 1) * P],
                                     wo_sb[:, ec * 512:(ec + 1) * 512],
                                     start=True, stop=True)
                    ob = small.tile([P, 512], f32, tag="ob")
                    nc.vector.tensor_copy(ob[:], op[:])
                    nc.sync.dma_start(
                        out_ext[rt * P:(rt + 1) * P, ec * 512:(ec + 1) * 512],
                        ob[:])

    nc.finalize()
    return nc


def _host_inputs():
    t = np.arange(32, dtype=np.float64)
    inv_freq = 1.0 / (ROPE_BASE ** (2.0 * t / DH))
    pos = np.arange(S, dtype=np.float64)
    ang = pos[None, :] * inv_freq[:, None]          # [32, S]
    cos32 = np.cos(ang).astype(np.float32)
    sin32 = np.sin(ang).astype(np.float32)
    cos32 = np.tile(cos32, (1, B))                  # [32, 4096]
    sin32 = np.tile(sin32, (1, B))
    cc = np.tile(cos32, (4, 1))                     # [128, 4096]
    ss = np.concatenate([-sin32, sin32, -sin32, sin32], axis=0)  # [128, 4096]

    ii = np.arange(P)[:, None]
    jj = np.arange(1024)[None, :]
    mask = (jj >= ii + 512).astype(ml_dtypes.bfloat16)   # [128, 1024]

    perm64 = np.concatenate([np.arange(0, 64, 2), np.arange(1, 64, 2)])
    return cc, ss, mask, perm64


def _in_maps(x, Wq, bq, Wk, bk, Wv, bv, Wo):
    cc, ss, mask, perm64 = _host_inputs()
    x2 = np.ascontiguousarray(x.reshape(NROWS, D))
    perm128 = np.concatenate([perm64, perm64 + 64])
    maps = []
    for c in range(8):
        sl = slice(c * CH, (c + 1) * CH)
        maps.append({
            "x": x2,
            "wqT": np.ascontiguousarray(Wq[sl][perm128].T).astype(ml_dtypes.bfloat16),
            "wkT": np.ascontiguousarray(Wk[sl][perm128].T).astype(ml_dtypes.bfloat16),
            "wvT": np.ascontiguousarray(Wv[sl].T).astype(ml_dtypes.bfloat16),
            "woT": np.ascontiguousarray(Wo[:, sl].T).astype(ml_dtypes.bfloat16),
            "bq": (bq[sl][perm128] * 0.125).reshape(CH, 1).copy(),
            "bk": bk[sl][perm128].reshape(CH, 1).copy(),
            "bv": bv[sl].reshape(CH, 1).copy(),
            "cc": cc, "ss": ss, "mask": mask,
        })
    return maps


def kernel(x, Wq, bq, Wk, bk, Wv, bv, Wo, bo):
    x = np.asarray(x, dtype=np.float32)
    Wq = np.asarray(Wq, dtype=np.float32)
    Wk = np.asarray(Wk, dtype=np.float32)
    Wv = np.asarray(Wv, dtype=np.float32)
    Wo = np.asarray(Wo, dtype=np.float32)
    bq = np.asarray(bq, dtype=np.float32)
    bk = np.asarray(bk, dtype=np.float32)
    bv = np.asarray(bv, dtype=np.float32)
    bo = np.asarray(bo, dtype=np.float32)

    if "nc" not in _CACHE:
        _CACHE["nc"] = _build()
    nc = _CACHE["nc"]

    res = run_bass_kernel_spmd(nc, _in_maps(x, Wq, bq, Wk, bk, Wv, bv, Wo),
                               core_ids=list(range(8)))
    out = np.zeros((NROWS, D), dtype=np.float32)
    for r in res.results:
        out += r["out"]
    out += bo[None, :]
    return out.reshape(B, S, D)


# revision 9
# speedup vs baseline: 1.3921x; 1.0125x over previous
"""Multi-head attention (B=2, S=2048, D=1024, H=16, causal, interleaved RoPE)
on 8 Trainium2 NeuronCores.

Sharding: tensor-parallel over heads — 2 heads (128 channels) per core.
Each core computes its Q/K/V projections, RoPE, causal attention, and a
row-parallel partial of the output projection; the host sums the partials.

Matmuls run in bf16 with fp32 PSUM accumulation (RoPE, softmax denominators
and all epilogues in fp32).

Layout:
  * Q/K projection weights are fed with output channels permuted so each
    head's dims are [evens(32), odds(32)] -> the RoPE pair-swap becomes a
    32-partition-block swap done with SBUF->SBUF DMAs; RoPE itself is three
    fp32 DVE multiplies/adds fused into the projection epilogue.
  * Attention uses the S^T layout: scores psum [k(128part), q(512)] via
    matmul(lhsT=K^T, rhs=Q^T), with the two heads issued back-to-back on
    disjoint PE row groups (partitions 0-63 / 64-127) so their weight loads
    overlap the other head's matmul. Softmax needs no max subtraction
    (scores are O(10)); exp on ACT writes bf16 P^T; causal masking is a
    multiply by a 0/1 slice of a [128,1024] band tile; PV via
    matmul(lhsT=V_aug, rhs=P^T) where V_aug carries a ones column so the
    denominator drops out as psum row 64; 1/denom (fast-approx reciprocal)
    is broadcast across partitions with a K=1 matmul and applied on the
    PV psum -> y^T copy.
  * x^T (contraction over D needs D on partitions) is produced on-device:
    cast to bf16 during the DMA (SWDGE), then PE transposes.
"""

import numpy as np
import ml_dtypes

import concourse.bacc as bacc
import concourse.mybir as mybir
import concourse.tile as tile
from concourse.bass_utils import run_bass_kernel_spmd
from concourse.masks import make_identity

P = 128
B, S, D = 2, 2048, 1024
H, DH = 16, 64
NROWS = B * S            # 4096 flattened rows
CH = 128                 # channels per core (2 heads)
RB = 512                 # row block for projections / q tiles
NRB = NROWS // RB        # 8
DSUB = D // P            # 8 contraction subtiles
KSUB = NROWS // P        # 32 k subtiles (128 rows each)
QT_PER_B = S // RB       # 4 q tiles per batch
ROPE_BASE = 10000.0

f32 = mybir.dt.float32
f32r = mybir.dt.float32r
bf16 = mybir.dt.bfloat16

_CACHE = {}


def _build():
    nc = bacc.Bacc("TRN2", target_bir_lowering=False)

    x_ext = nc.declare_dram_parameter("x", [NROWS, D], f32, isOutput=False)
    wqT_ext = nc.declare_dram_parameter("wqT", [D, CH], bf16, isOutput=False)
    wkT_ext = nc.declare_dram_parameter("wkT", [D, CH], bf16, isOutput=False)
    wvT_ext = nc.declare_dram_parameter("wvT", [D, CH], bf16, isOutput=False)
    woT_ext = nc.declare_dram_parameter("woT", [CH, D], bf16, isOutput=False)
    bq_ext = nc.declare_dram_parameter("bq", [CH, 1], f32, isOutput=False)
    bk_ext = nc.declare_dram_parameter("bk", [CH, 1], f32, isOutput=False)
    bv_ext = nc.declare_dram_parameter("bv", [CH, 1], f32, isOutput=False)
    cc_ext = nc.declare_dram_parameter("cc", [P, NROWS], f32, isOutput=False)
    ss_ext = nc.declare_dram_parameter("ss", [P, NROWS], f32, isOutput=False)
    mask_ext = nc.declare_dram_parameter("mask", [P, 1024], bf16, isOutput=False)
    out_ext = nc.declare_dram_parameter("out", [NROWS, D], f32, isOutput=True)

    with tile.TileContext(nc) as tc:
        with (
            tc.tile_pool(name="const", bufs=1) as cpool,
            tc.tile_pool(name="big", bufs=1) as big,
            tc.tile_pool(name="work", bufs=2) as work,
            tc.tile_pool(name="small", bufs=3) as small,
            tc.tile_pool(name="ptpool", bufs=6) as ptpool,
            tc.tile_pool(name="psumA", bufs=2, space="PSUM") as psumA,
            tc.tile_pool(name="psumB", bufs=1, space="PSUM") as psumB,
        ):
            # ---- constants ----
            ident_f = cpool.tile([P, P], f32, tag="identf")
            make_identity(nc, ident_f[:])
            ident = cpool.tile([P, P], bf16, tag="ident")
            nc.vector.tensor_copy(ident[:], ident_f[:])

            ones_f = cpool.tile([P, 64], f32, tag="onesf")
            nc.vector.memset(ones_f[:], 1.0)
            ones_b = cpool.tile([P, 64], bf16, tag="onesb")
            nc.vector.tensor_copy(ones_b[:], ones_f[:])
            ones_r = cpool.tile([P, 64], f32r, tag="onesr")
            nc.vector.tensor_copy(ones_r[:], ones_f[:])

            wq_sb = cpool.tile([P, DSUB, CH], bf16, tag="wq")
            wk_sb = cpool.tile([P, DSUB, CH], bf16, tag="wk")
            wv_sb = cpool.tile([P, DSUB, CH], bf16, tag="wv")
            for d in range(DSUB):
                nc.sync.dma_start(wq_sb[:, d], wqT_ext[d * P:(d + 1) * P, :])
                nc.sync.dma_start(wk_sb[:, d], wkT_ext[d * P:(d + 1) * P, :])
                nc.sync.dma_start(wv_sb[:, d], wvT_ext[d * P:(d + 1) * P, :])
            wo_sb = cpool.tile([CH, D], bf16, tag="wo")
            nc.sync.dma_start(wo_sb[:, 0:512], woT_ext[:, 0:512])
            nc.sync.dma_start(wo_sb[:, 512:1024], woT_ext[:, 512:1024])
            bq_sb = cpool.tile([CH, 1], f32, tag="bq")
            nc.sync.dma_start(bq_sb[:], bq_ext[:])
            bk_sb = cpool.tile([CH, 1], f32, tag="bk")
            nc.sync.dma_start(bk_sb[:], bk_ext[:])
            bv_sb = cpool.tile([CH, 1], f32, tag="bv")
            nc.sync.dma_start(bv_sb[:], bv_ext[:])
            mask_sb = cpool.tile([P, 1024], bf16, tag="mask")
            nc.sync.dma_start(mask_sb[:], mask_ext[:])

            # ---- persistent activation tiles ----
            qT = big.tile([P, NROWS], bf16, tag="qT")     # roped Q^T (pre-scaled 1/8)
            kT = big.tile([P, NROWS], bf16, tag="kT")     # roped K^T
            yT = big.tile([P, NROWS], bf16, tag="yT")     # attention out ^T
            v_sb = big.tile([P, KSUB, 130], bf16, tag="v")  # V natural + ones cols

            # ones columns of v (cols 64 and 129 of each k-subtile)
            nc.vector.tensor_copy(
                v_sb[:, :, 64:130:65].rearrange("p a b -> p (a b)"),
                ones_b[:, 0:2 * KSUB])

            # ====== phase A: x^T, projections, RoPE (fused per row block) ======
            proj_list = [
                ("q", wq_sb, bq_sb, 0.125, qT),
                ("k", wk_sb, bk_sb, 1.0, kT),
                ("v", wv_sb, bv_sb, 1.0, None),
            ]
            for rt in range(NRB):               # 8 blocks of 512 rows
                sl = slice(rt * RB, (rt + 1) * RB)
                xT = work.tile([P, DSUB, RB], bf16, tag="xT")
                for rc in range(RB // P):       # 4 chunks of 128 rows
                    r0 = rt * RB + rc * P
                    xab = work.tile([P, D], bf16, tag="xab")
                    # cast fp32 -> bf16 during the DMA (SWDGE)
                    nc.gpsimd.dma_start(xab[:], x_ext[r0:r0 + P, :])
                    for half in range(2):
                        tp = psumA.tile([P, 512], bf16, tag="tp")
                        for j in range(4):
                            d = half * 4 + j
                            nc.tensor.transpose(
                                tp[:, j * P:(j + 1) * P],
                                xab[:, d * P:(d + 1) * P], ident[:])
                        nc.vector.tensor_copy(
                            xT[:, half * 4:(half + 1) * 4, rc * P:(rc + 1) * P],
                            tp[:].rearrange("p (j c) -> p j c", j=4))

                ccc = small.tile([P, RB], f32, tag="ccc")
                nc.sync.dma_start(ccc[:], cc_ext[:, sl])
                sss = small.tile([P, RB], f32, tag="sss")
                nc.sync.dma_start(sss[:], ss_ext[:, sl])

                for name, w_sb, b_sb, scale, dstT in proj_list:
                    pp = psumA.tile([P, RB], f32, tag="proj")
                    for d in range(DSUB):
                        nc.tensor.matmul(pp[:], w_sb[:, d], xT[:, d],
                                         start=(d == 0), stop=(d == DSUB - 1))
                    if name != "v":
                        praw = work.tile([P, RB], f32, tag="praw")
                        nc.scalar.activation(
                            praw[:], pp[:],
                            mybir.ActivationFunctionType.Identity,
                            bias=b_sb[:, 0:1], scale=scale)
                        # RoPE: dst = praw*cc + swap32(praw)*ss  (fp32, ->bf16)
                        xsw = work.tile([P, RB], f32, tag="xsw")
                        for hh in range(2):
                            b0 = hh * 64
                            nc.sync.dma_start(xsw[b0:b0 + 32, :],
                                              praw[b0 + 32:b0 + 64, :])
                            nc.sync.dma_start(xsw[b0 + 32:b0 + 64, :],
                                              praw[b0:b0 + 32, :])
                        t1 = small.tile([P, RB], f32, tag="ropet1")
                        nc.vector.tensor_mul(t1[:], praw[:], ccc[:])
                        t2 = small.tile([P, RB], f32, tag="ropet2")
                        nc.vector.tensor_mul(t2[:], xsw[:], sss[:])
                        nc.vector.tensor_add(dstT[:, sl], t1[:], t2[:])
                    else:
                        # V^T chunk with bias, then PE-transpose to natural V
                        vr = work.tile([P, RB], bf16, tag="vraw")
                        nc.scalar.activation(
                            vr[:], pp[:],
                            mybir.ActivationFunctionType.Identity,
                            bias=b_sb[:, 0:1], scale=1.0)
                        tpv = psumA.tile([P, 512], bf16, tag="tp")
                        for rc2 in range(4):
                            nc.tensor.transpose(
                                tpv[:, rc2 * P:(rc2 + 1) * P],
                                vr[:, rc2 * P:(rc2 + 1) * P], ident[:])
                        # scatter: head0 chans -> cols 0:64, head1 -> cols 65:129
                        tpv_v = tpv[:].rearrange("p (k h c) -> p k h c", k=4, h=2)
                        vdst = (v_sb[:, rt * 4:(rt + 1) * 4, 0:130]
                                .rearrange("p k (h c) -> p k h c", h=2))
                        for hh in range(2):
                            nc.vector.tensor_copy(vdst[:, :, hh, 0:64],
                                                  tpv_v[:, :, hh, :])

            # ================= phase C: attention =================
            for b in range(B):
                for qt in range(QT_PER_B):
                    qcols = slice(b * S + qt * RB, b * S + (qt + 1) * RB)
                    nks = qt * 4 + 4
                    pvs = []
                    for h in range(2):
                        pv_t = psumB.tile([65, RB], f32, tag=f"pv{h}",
                                          name=f"pv{h}_{b}_{qt}")
                        pvs.append(pv_t)
                    for ks in range(nks):
                        kcols = slice(b * S + ks * P, b * S + (ks + 1) * P)
                        ksg = b * (S // P) + ks
                        m = ks - qt * 4
                        pts = []
                        for h in range(2):
                            hsl = slice(h * 64, (h + 1) * 64)
                            st = psumA.tile([P, RB], f32, tag="st")
                            nc.tensor.matmul(st[:], kT[hsl, kcols],
                                             qT[hsl, qcols],
                                             start=True, stop=True)
                            pt = ptpool.tile([P, RB], bf16, tag="pt")
                            nc.scalar.activation(pt[:], st[:],
                                                 mybir.ActivationFunctionType.Exp)
                            if m >= 0:
                                off = 512 - m * P
                                nc.vector.tensor_mul(pt[:], pt[:],
                                                     mask_sb[:, off:off + RB])
                            pts.append(pt)
                        for h in range(2):
                            nc.tensor.matmul(
                                pvs[h][:], v_sb[:, ksg, h * 65:(h + 1) * 65],
                                pts[h][:],
                                start=(ks == 0), stop=(ks == nks - 1))
                    for h in range(2):
                        pv = pvs[h]
                        rcp_f = small.tile([65, RB], f32, tag="rcpf")
                        with nc.allow_low_precision(reason="fp32 recip of fp32"):
                            nc.vector.reciprocal(rcp_f[64:65, :], pv[64:65, :])
                        rcp_r = small.tile([65, RB], f32r, tag="rcpr")
                        nc.vector.tensor_copy(rcp_r[64:65, :], rcp_f[64:65, :])
                        rep = psumA.tile([64, RB], f32, tag="tp")
                        nc.tensor.matmul(rep[:], ones_r[64:65, 0:64],
                                         rcp_r[64:65, :], start=True, stop=True)
                        rep_sb = small.tile([64, RB], f32, tag="repsb")
                        nc.scalar.copy(rep_sb[:], rep[:])
                        if h == 0:
                            nc.vector.tensor_mul(yT[0:64, qcols], pv[0:64, :],
                                                 rep_sb[:])
                        else:
                            t64 = small.tile([64, RB], bf16, tag="t64")
                            nc.vector.tensor_mul(t64[:], pv[0:64, :], rep_sb[:])
                            nc.sync.dma_start(yT[64:128, qcols], t64[:])

            # ================= phase D: output projection =================
            for rt in range(KSUB):              # 32 tiles of 128 rows
                for ec in range(2):
                    op = psumA.tile([P, 512], f32, tag="proj")
                    nc.tensor.matmul(op[:], yT[:, rt * P:(rt + 1) * P],
                                     wo_sb[:, ec * 512:(ec + 1) * 512],
                                     start=True, stop=True)
                    ob = small.tile([P, 512], f32, tag="ob")
                    nc.vector.tensor_copy(ob[:], op[:])
                    nc.sync.dma_start(
                        out_ext[rt * P:(rt + 1) * P, ec * 512:(ec + 1) * 512],
                        ob[:])

    nc.finalize()
    return nc


def _host_inputs():
    t = np.arange(32, dtype=np.float64)
    inv_freq = 1.0 / (ROPE_BASE ** (2.0 * t / DH))
    pos = np.arange(S, dtype=np.float64)
    ang = pos[None, :] * inv_freq[:, None]          # [32, S]
    cos32 = np.cos(ang).astype(np.float32)
    sin32 = np.sin(ang).astype(np.float32)
    cos32 = np.tile(cos32, (1, B))                  # [32, 4096]
    sin32 = np.tile(sin32, (1, B))
    cc = np.tile(cos32, (4, 1))                     # [128, 4096]
    ss = np.concatenate([-sin32, sin32, -sin32, sin32], axis=0)  # [128, 4096]

    ii = np.arange(P)[:, None]
    jj = np.arange(1024)[None, :]
    mask = (jj >= ii + 512).astype(ml_dtypes.bfloat16)   # [128, 1024]

    perm64 = np.concatenate([np.arange(0, 64, 2), np.arange(1, 64, 2)])
    return cc, ss, mask, perm64


def _in_maps(x, Wq, bq, Wk, bk, Wv, bv, Wo):
    cc, ss, mask, perm64 = _host_inputs()
    x2 = np.ascontiguousarray(x.reshape(NROWS, D))
    perm128 = np.concatenate([perm64, perm64 + 64])
    maps = []
    for c in range(8):
        sl = slice(c * CH, (c + 1) * CH)
        maps.append({
            "x": x2,
            "wqT": np.ascontiguousarray(Wq[sl][perm128].T).astype(ml_dtypes.bfloat16),
            "wkT": np.ascontiguousarray(Wk[sl][perm128].T).astype(ml_dtypes.bfloat16),
            "wvT": np.ascontiguousarray(Wv[sl].T).astype(ml_dtypes.bfloat16),
            "woT": np.ascontiguousarray(Wo[:, sl].T).astype(ml_dtypes.bfloat16),
            "bq": (bq[sl][perm128] * 0.125).reshape(CH, 1).copy(),
            "bk": bk[sl][perm128].reshape(CH, 1).copy(),
            "bv": bv[sl].reshape(CH, 1).copy(),
            "cc": cc, "ss": ss, "mask": mask,
        })
    return maps


def kernel(x, Wq, bq, Wk, bk, Wv, bv, Wo, bo):
    x = np.asarray(x, dtype=np.float32)
    Wq = np.asarray(Wq, dtype=np.float32)
    Wk = np.asarray(Wk, dtype=np.float32)
    Wv = np.asarray(Wv, dtype=np.float32)
    Wo = np.asarray(Wo, dtype=np.float32)
    bq = np.asarray(bq, dtype=np.float32)
    bk = np.asarray(bk, dtype=np.float32)
    bv = np.asarray(bv, dtype=np.float32)
    bo = np.asarray(bo, dtype=np.float32)

    if "nc" not in _CACHE:
        _CACHE["nc"] = _build()
    nc = _CACHE["nc"]

    res = run_bass_kernel_spmd(nc, _in_maps(x, Wq, bq, Wk, bk, Wv, bv, Wo),
                               core_ids=list(range(8)))
    out = np.zeros((NROWS, D), dtype=np.float32)
    for r in res.results:
        out += r["out"]
    out += bo[None, :]
    return out.reshape(B, S, D)
